# revision 2
# baseline (speedup 1.0000x reference)
"""Causal attention (B=8, S=2048, D=128, f32) on 8 TRN2 NeuronCores.

Strategy: batch-parallel SPMD — each core computes full causal attention for
one batch element.

Per-core algorithm (layouts chosen so softmax/PV need no on-chip transposes):
  - Host passes Q^T, K^T as [D=128, S=2048] bf16 (D on partitions) and V
    pre-arranged as VS [128, S] bf16 where column block j holds V rows
    [128j, 128j+128).
  - Scores are computed transposed, per key block j:
        S^T_j[k, q] = (K^T_j)-stationary.T @ Q^T-moving   (PSUM, f32)
  - exp with the 1/sqrt(D) scale folded into ScalarE's activation affine,
    PSUM -> SBUF, output in bf16 (P^T tiles). Diagonal blocks get a
    multiplicative causal mask (VectorE).
  - out^T[d, q] += V_j-stationary @ P^T_j-moving (bf16 in, f32 accumulate).
  - rowsum[q]: instead of a third full PE stream, consecutive P^T blocks
    are pair-summed IN PLACE on VectorE (pt_even += pt_odd over the
    overlap, bf16 2x mode) after PV consumed pt_even; the PE then streams
    only the pair-sums through a ones[128,128]-stationary matmul
    (PSUM-accumulated, rowsum replicated across partitions).  This halves
    the rowsum cost on both engines vs all-PE / all-DVE.
  - Normalize per 512-wide q-chunk as soon as its accumulation finishes:
    reciprocal_approx_fast on the PSUM rowsum, then multiply the out^T
    chunk directly from PSUM (no evacuation copy), DMA out on SP hardware
    queues.  The final chunk is split 2x256 to shorten the serial tail.
  - Host transposes out^T back to [S, D].

All DMAs ride SP hardware queues (parallel transfers, no software-DGE
drain at kernel end).  The PE clock ramp (HAM un-throttles after ~3.4us
of sustained activity) is started ASAP with a few dummy matmuls on
memset scratch while the first input DMAs are in flight; real score
matmuls take over as soon as kt/qt land.  The q axis is processed in two
passes of 1024 so PSUM fits:
  staging S^T [128,1024] x2 bufs (4 banks) + out^T [128,1024] (2 banks)
  + 2x rowsum [128,512] (2 banks) = 8 banks.
"""

import math
import sys

import numpy as np
import ml_dtypes

sys.path.insert(0, "/opt/trn_rl_repo")

from concourse import bacc, mybir
from concourse.bass_utils import run_bass_kernel_spmd
from concourse.tile import TileContext

F32 = mybir.dt.float32
BF16 = mybir.dt.bfloat16
BF16_NP = np.dtype(ml_dtypes.bfloat16)

B, S, D = 8, 2048, 128
NBLK = S // 128  # 16 key blocks
HALF = 1024  # q-pass width
SCALE = 1.0 / math.sqrt(D)

_NC_CACHE = None


def _chunks_for_block(j, q0):
    """Matmul chunks for key block j in pass [q0, q0+HALF): list of
    (a, b, h) global q ranges clipped to psum bank h (bf16: no min width)."""
    k0 = 128 * j
    q_lo = max(q0, k0)
    out = []
    for h in range(2):
        a = max(q_lo, q0 + 512 * h)
        b = q0 + 512 * (h + 1)
        if a < b:
            out.append((a, b, h))
    return out


def _build_nc():
    nc = bacc.Bacc("TRN2", target_bir_lowering=False, debug=False, num_devices=8)

    qt_d = nc.dram_tensor("QT", [D, S], BF16, kind="ExternalInput")
    kt_d = nc.dram_tensor("KT", [D, S], BF16, kind="ExternalInput")
    vs_d = nc.dram_tensor("VS", [128, S], BF16, kind="ExternalInput")
    out_d = nc.dram_tensor("out", [D, S], F32, kind="ExternalOutput")

    with TileContext(nc) as tc:
        with (
            tc.tile_pool(name="persist", bufs=1) as persist,
            tc.tile_pool(name="ptp", bufs=5) as ptp,
            tc.tile_pool(name="epi", bufs=2) as epi,
            tc.tile_pool(name="spool", bufs=2, space="PSUM") as spool,
            tc.tile_pool(name="opool", bufs=1, space="PSUM") as opool,
            tc.tile_pool(name="rpool", bufs=2, space="PSUM") as rpool,
        ):
            qt = persist.tile([D, S], BF16, tag="qt")
            kt = persist.tile([D, S], BF16, tag="kt")
            vs = persist.tile([128, S], BF16, tag="vs")  # col block j = V rows

            # warm the PE clock with dummy matmuls on memset scratch while
            # the first input DMAs are in flight; results are never read.
            # The memset is the first Pool instruction so the PE starts ASAP.
            pe_scr = persist.tile([128, 512], BF16, tag="pe_scr")
            nc.gpsimd.memset(pe_scr[:, :], 1.0)
            warm_ps = spool.tile([128, HALF], F32, tag="sps", name="warm_ps")
            for _w in range(4):
                nc.tensor.matmul(
                    warm_ps[:, 0:512],
                    pe_scr[:, 0:128],
                    pe_scr[:, :],
                    start=True,
                    stop=True,
                )

            # all-ones stationary for the rowsum partition-reduction (the
            # [128,128] ones stationary replicates the rowsum across all
            # output partitions, so no broadcast step is needed)
            ones_b = persist.tile([128, 128], BF16, tag="ones_b")
            nc.gpsimd.memset(ones_b[:, :], 1.0)

            # multiplicative causal mask, bf16: 1 where q >= k else 0
            mask = persist.tile([128, 128], BF16, tag="mask")
            nc.gpsimd.memset(mask[:, :], 1.0)
            nc.gpsimd.affine_select(
                out=mask[:, :],
                in_=mask[:, :],
                compare_op=mybir.AluOpType.is_ge,
                fill=0.0,
                base=0,
                pattern=[[1, 128]],
                channel_multiplier=-1,
            )

            # warm the ScalarE exp table while input DMAs run
            warm_src = persist.tile([1, 16], F32, tag="warm_src")
            nc.gpsimd.memset(warm_src[:, :], 0.0)
            warm = epi.tile([1, 16], F32, tag="warm")
            nc.scalar.activation(
                warm[:, :],
                warm_src[:, :],
                mybir.ActivationFunctionType.Exp,
                scale=SCALE,
            )

            # ---- input DMAs: all on SP hardware queues (parallel
            # transfers, ~600ns issue each, serialized on the Sync queue).
            # Ordered so the score pipeline (kt, qt) never starves; V
            # chunks land comfortably before their PV consumers.
            def sdma(dst, src):
                nc.sync.dma_start(dst, src)

            sdma(kt[:, 0:128], kt_d[:, 0:128])
            sdma(qt[:, 0:512], qt_d[:, 0:512])
            sdma(kt[:, 128:512], kt_d[:, 128:512])
            sdma(qt[:, 512:1024], qt_d[:, 512:1024])
            sdma(kt[:, 512:1024], kt_d[:, 512:1024])
            sdma(vs[:, 0:512], vs_d[:, 0:512])
            sdma(qt[:, 1024:1536], qt_d[:, 1024:1536])
            sdma(qt[:, 1536:2048], qt_d[:, 1536:2048])
            sdma(kt[:, 1024:2048], kt_d[:, 1024:2048])
            sdma(vs[:, 512:1024], vs_d[:, 512:1024])
            sdma(vs[:, 1024:1536], vs_d[:, 1024:1536])
            sdma(vs[:, 1536:2048], vs_d[:, 1536:2048])

            pts = {}

            def emit_scores(qh, j):
                """QK^T chunks + per-block exp (+ causal mask on the
                diagonal block) for key block j.  Pass-independent so the
                two-deep score pipeline can run across the pass boundary."""
                q0 = qh * HALF
                k0 = 128 * j
                chunks = _chunks_for_block(j, q0)

                sps = spool.tile([128, HALF], F32, tag="sps",
                                 name=f"sps_{qh}_{j}")
                pt = ptp.tile([128, HALF], BF16, tag="pt",
                              name=f"pt_{qh}_{j}")
                for (a, b, _h) in chunks:
                    nc.tensor.matmul(
                        sps[:, a - q0 : b - q0],
                        kt[:, k0 : k0 + 128],
                        qt[:, a:b],
                        start=True,
                        stop=True,
                    )
                lo = chunks[0][0]
                nc.scalar.activation(
                    pt[:, lo - q0 : HALF],
                    sps[:, lo - q0 : HALF],
                    mybir.ActivationFunctionType.Exp,
                    scale=SCALE,
                )
                if k0 >= q0:
                    dl = k0 - q0
                    nc.vector.tensor_mul(
                        pt[:, dl : dl + 128],
                        pt[:, dl : dl + 128],
                        mask[:, :],
                    )
                pts[(qh, j)] = pt

            emit_scores(0, 0)
            emit_scores(0, 1)

            for qh in range(2):
                q0 = qh * HALF  # global q offset of this pass
                njb = (q0 + HALF) // 128  # key blocks this pass
                npair = njb // 2

                out_ps = opool.tile([D, HALF], F32, tag="outps",
                                    name=f"outps_{qh}")
                rs = [
                    rpool.tile([128, 512], F32, tag="rs", name=f"rs_{qh}_{h}")
                    for h in range(2)
                ]
                # last key block / pair that touches each 512-half
                j_last = [(q0 + 512 * (h + 1)) // 128 - 1 for h in range(2)]
                m_last = [(q0 + 512 * (h + 1)) // 256 - 1 for h in range(2)]

                def emit_pv(j, pt, q0=q0, j_last=j_last):
                    """PV accumulation for key block j."""
                    k0 = 128 * j
                    for (a, b, h) in _chunks_for_block(j, q0):
                        nc.tensor.matmul(
                            out_ps[:, a - q0 : b - q0],
                            vs[:, k0 : k0 + 128],
                            pt[:, a - q0 : b - q0],
                            start=(j == 0),
                            stop=(j == j_last[h]),
                        )

                def emit_pair_add(m, q0=q0):
                    """pt_even += pt_odd over the causal overlap (VectorE,
                    bf16 2x).  Runs after PV consumed pt_even."""
                    pa = pts[(qh, 2 * m)]
                    pb = pts[(qh, 2 * m + 1)]
                    lo = max(q0, 128 * (2 * m + 1)) - q0
                    nc.vector.tensor_add(
                        pa[:, lo:HALF], pa[:, lo:HALF], pb[:, lo:HALF]
                    )

                def emit_rowsum(m, q0=q0, rs=rs, m_last=m_last):
                    """Stream the pair-sum through the ones stationary into
                    the PSUM rowsum accumulator (per 512-chunk)."""
                    pa = pts[(qh, 2 * m)]
                    su = max(q0, 256 * m)
                    for h in range(2):
                        a = max(su, q0 + 512 * h)
                        b = q0 + 512 * (h + 1)
                        if a >= b:
                            continue
                        nc.tensor.matmul(
                            rs[h][:, a - (q0 + 512 * h) : b - (q0 + 512 * h)],
                            ones_b[:, :],
                            pa[:, a - q0 : b - q0],
                            start=(m == 0),
                            stop=(m == m_last[h]),
                        )

                def emit_epi_half(h, qh=qh, q0=q0, rs=rs, out_ps=out_ps):
                    """Normalize + store q-chunk [q0+512h, q0+512h+512).
                    Multiplies straight out of PSUM — no evacuation copy."""
                    rb = epi.tile([128, 512], F32, tag="rb",
                                  name=f"rb_{qh}_{h}")
                    o_fin = epi.tile([D, 512], F32, tag="o_fin",
                                     name=f"ofin_{qh}_{h}")
                    if qh == 1 and h == 1:
                        # final tail: 2x256 pipeline to shorten the serial
                        # recip->mul->store chain after the last matmul
                        for c in range(2):
                            cs = slice(256 * c, 256 * (c + 1))
                            nc.vector.reciprocal_approx_fast(
                                out=rb[:, cs], in_=rs[h][:, cs]
                            )
                            nc.vector.tensor_mul(
                                o_fin[:, cs],
                                out_ps[:, 512 * h + 256 * c :
                                       512 * h + 256 * (c + 1)],
                                rb[:, cs],
                            )
                            nc.sync.dma_start(
                                out_d[:, q0 + 512 * h + 256 * c :
                                      q0 + 512 * h + 256 * (c + 1)],
                                o_fin[:, cs],
                            )
                    else:
                        nc.vector.reciprocal_approx_fast(
                            out=rb[:, :], in_=rs[h][:, :]
                        )
                        nc.vector.tensor_mul(
                            o_fin[:, :],
                            out_ps[:, 512 * h : 512 * (h + 1)],
                            rb[:, :],
                        )
                        for c in range(2):
                            nc.sync.dma_start(
                                out_d[:, q0 + 512 * h + 256 * c :
                                      q0 + 512 * h + 256 * (c + 1)],
                                o_fin[:, 256 * c : 256 * (c + 1)],
                            )

                # software pipeline, two-deep on the consume side and
                # continued ACROSS the pass boundary: pass-1's first two
                # score groups are emitted during pass-0's last iterations
                for j in range(njb):
                    nj = j + 2
                    if nj < njb:
                        emit_scores(qh, nj)
                    elif qh == 0:
                        emit_scores(1, nj - njb)
                    emit_pv(j, pts[(qh, j)])
                    if j % 2 == 1:
                        m = j // 2
                        emit_pair_add(m)
                        emit_rowsum(m)
                        pts.pop((qh, 2 * m))
                        pts.pop((qh, 2 * m + 1))
                    for h in range(2):
                        if j == j_last[h]:
                            emit_epi_half(h)

    nc.compile()
    return nc


def _get_nc():
    global _NC_CACHE
    if _NC_CACHE is None:
        _NC_CACHE = _build_nc()
    return _NC_CACHE


def _in_maps(Q, K, V):
    maps = []
    for b in range(B):
        vsb = np.ascontiguousarray(
            V[b].reshape(NBLK, 128, D).transpose(1, 0, 2).reshape(128, S)
        ).astype(BF16_NP)
        maps.append(
            {
                "QT": np.ascontiguousarray(Q[b].T).astype(BF16_NP),
                "KT": np.ascontiguousarray(K[b].T).astype(BF16_NP),
                "VS": vsb,
            }
        )
    return maps


def kernel(Q, K, V):
    Q = np.asarray(Q, dtype=np.float32)
    K = np.asarray(K, dtype=np.float32)
    V = np.asarray(V, dtype=np.float32)
    assert Q.shape == (B, S, D), Q.shape

    nc = _get_nc()
    res = run_bass_kernel_spmd(nc, _in_maps(Q, K, V), core_ids=list(range(B)))
    return np.stack(
        [np.ascontiguousarray(res.results[b]["out"].T) for b in range(B)], axis=0
    )


# revision 7
# speedup vs baseline: 1.0483x; 1.0483x over previous
"""Causal attention (B=8, S=2048, D=128, f32) on 8 TRN2 NeuronCores.

Strategy: batch-parallel SPMD — each core computes full causal attention for
one batch element.

Per-core algorithm (layouts chosen so softmax/PV need no on-chip transposes):
  - Host passes Q^T, K^T as [D=128, S=2048] bf16 (D on partitions) and V
    pre-arranged as VS [128, S] bf16 where column block j holds V rows
    [128j, 128j+128).
  - Scores are computed transposed, per key block j:
        S^T_j[k, q] = (K^T_j)-stationary.T @ Q^T-moving   (PSUM, f32)
  - exp with the 1/sqrt(D) scale folded into ScalarE's activation affine,
    PSUM -> SBUF, output in bf16 (P^T tiles). Diagonal blocks get a
    multiplicative causal mask (VectorE).
  - out^T[d, q] += V_j-stationary @ P^T_j-moving (bf16 in, f32 accumulate).
  - rowsum[q]: instead of a third full PE stream, consecutive P^T blocks
    are pair-summed IN PLACE on VectorE (pt_even += pt_odd over the
    overlap, bf16 2x mode) after PV consumed pt_even; the PE then streams
    only the pair-sums through a ones[128,128]-stationary matmul
    (PSUM-accumulated, rowsum replicated across partitions).  This halves
    the rowsum cost on both engines vs all-PE / all-DVE.
  - Normalize per 512-wide q-chunk as soon as its accumulation finishes:
    reciprocal_approx_fast on the PSUM rowsum, then multiply the out^T
    chunk directly from PSUM (no evacuation copy), DMA out on SP hardware
    queues.  The final chunk is split 2x256 to shorten the serial tail.
  - Host transposes out^T back to [S, D].

All DMAs ride SP hardware queues (parallel transfers, no software-DGE
drain at kernel end).  The PE clock ramp (HAM un-throttles after ~3.4us
of sustained activity) is started ASAP with a few dummy matmuls on
memset scratch while the first input DMAs are in flight; real score
matmuls take over as soon as kt/qt land.  The q axis is processed in two
passes of 1024 so PSUM fits:
  staging S^T [128,1024] x2 bufs (4 banks) + out^T [128,1024] (2 banks)
  + 2x rowsum [128,512] (2 banks) = 8 banks.
"""

import math
import sys

import numpy as np
import ml_dtypes

sys.path.insert(0, "/opt/trn_rl_repo")

from concourse import bacc, mybir
from concourse.bass_utils import run_bass_kernel_spmd
from concourse.tile import TileContext

F32 = mybir.dt.float32
BF16 = mybir.dt.bfloat16
BF16_NP = np.dtype(ml_dtypes.bfloat16)

B, S, D = 8, 2048, 128
NBLK = S // 128  # 16 key blocks
HALF = 1024  # q-pass width
SCALE = 1.0 / math.sqrt(D)

_NC_CACHE = None


def _chunks_for_block(j, q0):
    """Matmul chunks for key block j in pass [q0, q0+HALF): list of
    (a, b, h) global q ranges clipped to psum bank h (bf16: no min width)."""
    k0 = 128 * j
    q_lo = max(q0, k0)
    out = []
    for h in range(2):
        a = max(q_lo, q0 + 512 * h)
        b = q0 + 512 * (h + 1)
        if a < b:
            out.append((a, b, h))
    return out


def _build_nc():
    nc = bacc.Bacc("TRN2", target_bir_lowering=False, debug=False, num_devices=8)

    qt_d = nc.dram_tensor("QT", [D, S], BF16, kind="ExternalInput")
    kt_d = nc.dram_tensor("KT", [D, S], BF16, kind="ExternalInput")
    vs_d = nc.dram_tensor("VS", [128, S], BF16, kind="ExternalInput")
    out_d = nc.dram_tensor("out", [D, S], F32, kind="ExternalOutput")

    with TileContext(nc) as tc:
        with (
            tc.tile_pool(name="persist", bufs=1) as persist,
            tc.tile_pool(name="ptp", bufs=5) as ptp,
            tc.tile_pool(name="epi", bufs=2) as epi,
            tc.tile_pool(name="spool", bufs=2, space="PSUM") as spool,
            tc.tile_pool(name="opool", bufs=1, space="PSUM") as opool,
            tc.tile_pool(name="rpool", bufs=2, space="PSUM") as rpool,
        ):
            qt = persist.tile([D, S], BF16, tag="qt")
            kt = persist.tile([D, S], BF16, tag="kt")
            vs = persist.tile([128, S], BF16, tag="vs")  # col block j = V rows

            # warm the PE clock with dummy matmuls on memset scratch while
            # the first input DMAs are in flight; results are never read.
            # The memset is the first Pool instruction so the PE starts ASAP.
            pe_scr = persist.tile([128, 512], BF16, tag="pe_scr")
            nc.gpsimd.memset(pe_scr[:, :], 1.0)
            warm_ps = spool.tile([128, HALF], F32, tag="sps", name="warm_ps")
            for _w in range(7):
                nc.tensor.matmul(
                    warm_ps[:, 0:512],
                    pe_scr[:, 0:128],
                    pe_scr[:, :],
                    start=True,
                    stop=True,
                )

            # all-ones stationary for the rowsum partition-reduction (the
            # [128,128] ones stationary replicates the rowsum across all
            # output partitions, so no broadcast step is needed)
            ones_b = persist.tile([128, 128], BF16, tag="ones_b")
            nc.gpsimd.memset(ones_b[:, :], 1.0)

            # multiplicative causal mask, bf16: 1 where q >= k else 0
            mask = persist.tile([128, 128], BF16, tag="mask")
            nc.gpsimd.memset(mask[:, :], 1.0)
            nc.gpsimd.affine_select(
                out=mask[:, :],
                in_=mask[:, :],
                compare_op=mybir.AluOpType.is_ge,
                fill=0.0,
                base=0,
                pattern=[[1, 128]],
                channel_multiplier=-1,
            )

            # ---- input DMAs: hardware DGE queues only (SP + ScalarE),
            # parallel transfers, no software-DGE drain at kernel end.
            # The first two issues ride different engine queues so qt/kt
            # land in parallel; ordered so the score pipeline never
            # starves and V chunks land before their PV consumers.
            nc.scalar.dma_start(kt[:, 0:256], kt_d[:, 0:256])
            nc.sync.dma_start(qt[:, 0:512], qt_d[:, 0:512])
            nc.sync.dma_start(kt[:, 256:1024], kt_d[:, 256:1024])
            nc.sync.dma_start(qt[:, 512:1024], qt_d[:, 512:1024])
            nc.sync.dma_start(vs[:, 0:512], vs_d[:, 0:512])
            nc.sync.dma_start(qt[:, 1024:2048], qt_d[:, 1024:2048])
            nc.sync.dma_start(kt[:, 1024:2048], kt_d[:, 1024:2048])
            nc.sync.dma_start(vs[:, 512:1024], vs_d[:, 512:1024])
            nc.sync.dma_start(vs[:, 1024:1536], vs_d[:, 1024:1536])
            nc.sync.dma_start(vs[:, 1536:2048], vs_d[:, 1536:2048])

            # warm the ScalarE exp table (the auto-inserted table load
            # runs right after the kt head DMA issue) while inputs land
            warm_src = persist.tile([1, 16], F32, tag="warm_src")
            nc.gpsimd.memset(warm_src[:, :], 0.0)
            warm = epi.tile([1, 16], F32, tag="warm")
            nc.scalar.activation(
                warm[:, :],
                warm_src[:, :],
                mybir.ActivationFunctionType.Exp,
                scale=SCALE,
            )

            pts = {}

            def emit_scores(qh, j):
                """QK^T chunks + per-block exp (+ causal mask on the
                diagonal block) for key block j.  Pass-independent so the
                two-deep score pipeline can run across the pass boundary."""
                q0 = qh * HALF
                k0 = 128 * j
                chunks = _chunks_for_block(j, q0)

                sps = spool.tile([128, HALF], F32, tag="sps",
                                 name=f"sps_{qh}_{j}")
                pt = ptp.tile([128, HALF], BF16, tag="pt",
                              name=f"pt_{qh}_{j}")
                for (a, b, _h) in chunks:
                    nc.tensor.matmul(
                        sps[:, a - q0 : b - q0],
                        kt[:, k0 : k0 + 128],
                        qt[:, a:b],
                        start=True,
                        stop=True,
                    )
                lo = chunks[0][0]
                nc.scalar.activation(
                    pt[:, lo - q0 : HALF],
                    sps[:, lo - q0 : HALF],
                    mybir.ActivationFunctionType.Exp,
                    scale=SCALE,
                )
                if k0 >= q0:
                    dl = k0 - q0
                    nc.vector.tensor_mul(
                        pt[:, dl : dl + 128],
                        pt[:, dl : dl + 128],
                        mask[:, :],
                    )
                pts[(qh, j)] = pt

            emit_scores(0, 0)
            emit_scores(0, 1)

            for qh in range(2):
                q0 = qh * HALF  # global q offset of this pass
                njb = (q0 + HALF) // 128  # key blocks this pass
                npair = njb // 2

                out_ps = opool.tile([D, HALF], F32, tag="outps",
                                    name=f"outps_{qh}")
                rs = [
                    rpool.tile([128, 512], F32, tag="rs", name=f"rs_{qh}_{h}")
                    for h in range(2)
                ]
                # last key block / pair that touches each 512-half
                j_last = [(q0 + 512 * (h + 1)) // 128 - 1 for h in range(2)]
                m_last = [(q0 + 512 * (h + 1)) // 256 - 1 for h in range(2)]

                def emit_pv(j, pt, q0=q0, j_last=j_last):
                    """PV accumulation for key block j."""
                    k0 = 128 * j
                    for (a, b, h) in _chunks_for_block(j, q0):
                        nc.tensor.matmul(
                            out_ps[:, a - q0 : b - q0],
                            vs[:, k0 : k0 + 128],
                            pt[:, a - q0 : b - q0],
                            start=(j == 0),
                            stop=(j == j_last[h]),
                        )

                def emit_pair_add(m, q0=q0):
                    """pt_even += pt_odd over the causal overlap (VectorE,
                    bf16 2x).  Runs after PV consumed pt_even."""
                    pa = pts[(qh, 2 * m)]
                    pb = pts[(qh, 2 * m + 1)]
                    lo = max(q0, 128 * (2 * m + 1)) - q0
                    nc.vector.tensor_add(
                        pa[:, lo:HALF], pa[:, lo:HALF], pb[:, lo:HALF]
                    )

                def emit_rowsum(m, q0=q0, rs=rs, m_last=m_last):
                    """Stream the pair-sum through the ones stationary into
                    the PSUM rowsum accumulator (per 512-chunk)."""
                    pa = pts[(qh, 2 * m)]
                    su = max(q0, 256 * m)
                    for h in range(2):
                        a = max(su, q0 + 512 * h)
                        b = q0 + 512 * (h + 1)
                        if a >= b:
                            continue
                        nc.tensor.matmul(
                            rs[h][:, a - (q0 + 512 * h) : b - (q0 + 512 * h)],
                            ones_b[:, :],
                            pa[:, a - q0 : b - q0],
                            start=(m == 0),
                            stop=(m == m_last[h]),
                        )

                def emit_epi_half(h, qh=qh, q0=q0, rs=rs, out_ps=out_ps):
                    """Normalize + store q-chunk [q0+512h, q0+512h+512).
                    Multiplies straight out of PSUM — no evacuation copy."""
                    rb = epi.tile([128, 512], F32, tag="rb",
                                  name=f"rb_{qh}_{h}")
                    o_fin = epi.tile([D, 512], F32, tag="o_fin",
                                     name=f"ofin_{qh}_{h}")
                    if qh == 1 and h == 1:
                        # final tail: 2x256 pipeline to shorten the serial
                        # recip->mul->store chain after the last matmul;
                        # the two store issues ride different hardware-DGE
                        # queues (ScalarE is idle by now) so they overlap
                        for c in range(2):
                            cs = slice(256 * c, 256 * (c + 1))
                            nc.vector.reciprocal_approx_fast(
                                out=rb[:, cs], in_=rs[h][:, cs]
                            )
                            nc.vector.tensor_mul(
                                o_fin[:, cs],
                                out_ps[:, 512 * h + 256 * c :
                                       512 * h + 256 * (c + 1)],
                                rb[:, cs],
                            )
                            eng = nc.scalar if c == 0 else nc.sync
                            eng.dma_start(
                                out_d[:, q0 + 512 * h + 256 * c :
                                      q0 + 512 * h + 256 * (c + 1)],
                                o_fin[:, cs],
                            )
                    else:
                        nc.vector.reciprocal_approx_fast(
                            out=rb[:, :], in_=rs[h][:, :]
                        )
                        nc.vector.tensor_mul(
                            o_fin[:, :],
                            out_ps[:, 512 * h : 512 * (h + 1)],
                            rb[:, :],
                        )
                        for c in range(2):
                            nc.sync.dma_start(
                                out_d[:, q0 + 512 * h + 256 * c :
                                      q0 + 512 * h + 256 * (c + 1)],
                                o_fin[:, 256 * c : 256 * (c + 1)],
                            )

                # software pipeline, two-deep on the consume side and
                # continued ACROSS the pass boundary: pass-1's first two
                # score groups are emitted during pass-0's last iterations
                for j in range(njb):
                    nj = j + 2
                    if nj < njb:
                        emit_scores(qh, nj)
                    elif qh == 0:
                        emit_scores(1, nj - njb)
                    emit_pv(j, pts[(qh, j)])
                    if j % 2 == 1:
                        m = j // 2
                        emit_pair_add(m)
                        emit_rowsum(m)
                        pts.pop((qh, 2 * m))
                        pts.pop((qh, 2 * m + 1))
                    for h in range(2):
                        if j == j_last[h]:
                            emit_epi_half(h)

    nc.compile()
    return nc


def _get_nc():
    global _NC_CACHE
    if _NC_CACHE is None:
        _NC_CACHE = _build_nc()
    return _NC_CACHE


def _in_maps(Q, K, V):
    maps = []
    for b in range(B):
        vsb = np.ascontiguousarray(
            V[b].reshape(NBLK, 128, D).transpose(1, 0, 2).reshape(128, S)
        ).astype(BF16_NP)
        maps.append(
            {
                "QT": np.ascontiguousarray(Q[b].T).astype(BF16_NP),
                "KT": np.ascontiguousarray(K[b].T).astype(BF16_NP),
                "VS": vsb,
            }
        )
    return maps


def kernel(Q, K, V):
    Q = np.asarray(Q, dtype=np.float32)
    K = np.asarray(K, dtype=np.float32)
    V = np.asarray(V, dtype=np.float32)
    assert Q.shape == (B, S, D), Q.shape

    nc = _get_nc()
    res = run_bass_kernel_spmd(nc, _in_maps(Q, K, V), core_ids=list(range(B)))
    return np.stack(
        [np.ascontiguousarray(res.results[b]["out"].T) for b in range(B)], axis=0
    )


# revision 9
# speedup vs baseline: 1.1969x; 1.1418x over previous
"""Causal attention (B=8, S=2048, D=128, f32) on 8 TRN2 NeuronCores.

Strategy: batch-parallel SPMD — each core computes full causal attention for
one batch element.

Per-core algorithm (layouts chosen so softmax/PV need no on-chip transposes):
  - Host passes Q^T, K^T as [D=128, S=2048] bf16 (D on partitions) and V
    pre-arranged as VS [128, S] bf16 where column block j holds V rows
    [128j, 128j+128).
  - Scores are computed transposed, per key block j:
        S^T_j[k, q] = (K^T_j)-stationary.T @ Q^T-moving   (PSUM, f32)
  - exp with the 1/sqrt(D) scale folded into ScalarE's activation affine,
    PSUM -> SBUF, output in bf16 (P^T tiles).  Diagonal blocks get a
    multiplicative causal mask (VectorE).  The ScalarE exp stream is the
    critical resource (~14.5us of columns + ~0.3us/instr overhead), so
    the narrow tail blocks of each pass are packed in PAIRS into one
    scores tile and exp'd with a single activation.
  - out^T[d, q] += V_j-stationary @ P^T_j-moving (bf16 in, f32 accumulate).
  - rowsum[q]: consecutive P^T blocks are pair-summed IN PLACE on VectorE
    (pt_even += pt_odd over the overlap, bf16) after PV consumed pt_even;
    the PE streams only the pair-sums through a ones[128,128]-stationary
    matmul (PSUM-accumulated, rowsum replicated across partitions).  This
    halves the rowsum cost on both engines vs all-PE / all-DVE.
  - Normalize per 512-wide q-chunk as soon as its accumulation finishes:
    reciprocal_approx_fast on the PSUM rowsum, then multiply the out^T
    chunk directly from PSUM (no evacuation copy), DMA out on hardware
    DGE queues.  The final chunk is split (384,128) to shorten the
    serial recip->mul->store tail after the last matmul.
  - Host transposes out^T back to [S, D].

All DMAs ride hardware DGE queues (SP + ScalarE; parallel transfers, no
software-DGE drain at kernel end).  The PE clock ramp (HAM un-throttles
after ~3us of *continuous* activity — any >1us idle gap restarts the
clock) is covered by dummy matmuls on memset scratch while the first
input DMAs are in flight, plus two fillers over the score-pipeline fill
bubble.  The q axis is processed in two passes of 1024 so PSUM fits:
  staging S^T [128,1024] x2 bufs (4 banks) + out^T [128,1024] (2 banks)
  + 2x rowsum [128,512] (2 banks) = 8 banks.
"""

import math
import sys

import numpy as np
import ml_dtypes

sys.path.insert(0, "/opt/trn_rl_repo")

from concourse import bacc, mybir
from concourse.bass_utils import run_bass_kernel_spmd
from concourse.tile import TileContext

F32 = mybir.dt.float32
BF16 = mybir.dt.bfloat16
BF16_NP = np.dtype(ml_dtypes.bfloat16)

B, S, D = 8, 2048, 128
NBLK = S // 128  # 16 key blocks
HALF = 1024  # q-pass width
SCALE = 1.0 / math.sqrt(D)

_NC_CACHE = None


def _build_nc():
    nc = bacc.Bacc("TRN2", target_bir_lowering=False, debug=False, num_devices=8)

    qt_d = nc.dram_tensor("QT", [D, S], BF16, kind="ExternalInput")
    kt_d = nc.dram_tensor("KT", [D, S], BF16, kind="ExternalInput")
    vs_d = nc.dram_tensor("VS", [128, S], BF16, kind="ExternalInput")
    out_d = nc.dram_tensor("out", [D, S], F32, kind="ExternalOutput")

    with TileContext(nc) as tc:
        with (
            tc.tile_pool(name="persist", bufs=1) as persist,
            tc.tile_pool(name="ptp", bufs=5) as ptp,
            tc.tile_pool(name="epi", bufs=2) as epi,
            tc.tile_pool(name="spool", bufs=2, space="PSUM") as spool,
            tc.tile_pool(name="opool", bufs=1, space="PSUM") as opool,
            tc.tile_pool(name="rpool", bufs=2, space="PSUM") as rpool,
        ):
            qt = persist.tile([D, S], BF16, tag="qt")
            kt = persist.tile([D, S], BF16, tag="kt")
            vs = persist.tile([128, S], BF16, tag="vs")  # col block j = V rows

            # warm the PE clock with dummy matmuls on memset scratch while
            # the first input DMAs are in flight; results are never read.
            # The memset is the first Pool instruction so the PE starts ASAP.
            pe_scr = persist.tile([128, 512], BF16, tag="pe_scr")
            nc.gpsimd.memset(pe_scr[:, :], 1.0)
            warm_ps = spool.tile([128, HALF], F32, tag="sps", name="warm_ps")
            for _w in range(7):
                nc.tensor.matmul(
                    warm_ps[:, 0:512],
                    pe_scr[:, 0:128],
                    pe_scr[:, :],
                    start=True,
                    stop=True,
                )
            # scratch for the two fill-bubble warm matmuls (the real rowsum
            # stream re-starts this accumulator with start=True later)
            warm_rs = rpool.tile([128, 512], F32, tag="rs", name="warm_rs")

            # all-ones stationary for the rowsum partition-reduction (the
            # [128,128] ones stationary replicates the rowsum across all
            # output partitions, so no broadcast step is needed)
            ones_b = persist.tile([128, 128], BF16, tag="ones_b")
            nc.gpsimd.memset(ones_b[:, :], 1.0)

            # multiplicative causal mask, bf16: 1 where q >= k else 0
            mask = persist.tile([128, 128], BF16, tag="mask")
            nc.gpsimd.memset(mask[:, :], 1.0)
            nc.gpsimd.affine_select(
                out=mask[:, :],
                in_=mask[:, :],
                compare_op=mybir.AluOpType.is_ge,
                fill=0.0,
                base=0,
                pattern=[[1, 128]],
                channel_multiplier=-1,
            )

            # ---- input DMAs: hardware DGE queues only (SP + ScalarE),
            # parallel transfers, no software-DGE drain at kernel end.
            # The first two issues ride different engine queues so qt/kt
            # land in parallel; ordered so the score pipeline never
            # starves and V chunks land before their PV consumers.
            nc.scalar.dma_start(kt[:, 0:256], kt_d[:, 0:256])
            nc.sync.dma_start(qt[:, 0:512], qt_d[:, 0:512])
            nc.sync.dma_start(kt[:, 256:1024], kt_d[:, 256:1024])
            nc.sync.dma_start(qt[:, 512:1024], qt_d[:, 512:1024])
            nc.sync.dma_start(vs[:, 0:512], vs_d[:, 0:512])
            nc.sync.dma_start(qt[:, 1024:2048], qt_d[:, 1024:2048])
            nc.sync.dma_start(kt[:, 1024:2048], kt_d[:, 1024:2048])
            nc.sync.dma_start(vs[:, 512:1024], vs_d[:, 512:1024])
            nc.sync.dma_start(vs[:, 1024:1536], vs_d[:, 1024:1536])
            nc.sync.dma_start(vs[:, 1536:2048], vs_d[:, 1536:2048])

            # warm the ScalarE exp table (the auto-inserted table load
            # runs right after the kt head DMA issue) while inputs land
            warm_src = persist.tile([1, 16], F32, tag="warm_src")
            nc.gpsimd.memset(warm_src[:, :], 0.0)
            warm = epi.tile([1, 16], F32, tag="warm")
            nc.scalar.activation(
                warm[:, :],
                warm_src[:, :],
                mybir.ActivationFunctionType.Exp,
                scale=SCALE,
            )

            # pts[(qh, j)] = (tile, shift): P^T for within-pass q-index x
            # (x in [lo_j, HALF)) lives at tile[:, x - shift].
            pts = {}

            def emit_group(qh, j0, nb):
                """Scores + one exp for blocks j0..j0+nb-1 of pass qh.
                nb=1: standard layout (shift 0).  nb=2: the two blocks are
                packed back-to-back in one tile ([0,w_a) and [w_a,w_a+w_b))
                and exp'd with a single activation."""
                q0 = qh * HALF
                sps = spool.tile([128, HALF], F32, tag="sps",
                                 name=f"sps_{qh}_{j0}")
                pt = ptp.tile([128, HALF], BF16, tag="pt",
                              name=f"pt_{qh}_{j0}")
                if nb == 1:
                    k0 = 128 * j0
                    q_lo = max(q0, k0)
                    for h in range(2):
                        a = max(q_lo, q0 + 512 * h)
                        b = q0 + 512 * (h + 1)
                        if a < b:
                            nc.tensor.matmul(
                                sps[:, a - q0 : b - q0],
                                kt[:, k0 : k0 + 128],
                                qt[:, a:b],
                                start=True,
                                stop=True,
                            )
                    lo = q_lo - q0
                    nc.scalar.activation(
                        pt[:, lo:HALF],
                        sps[:, lo:HALF],
                        mybir.ActivationFunctionType.Exp,
                        scale=SCALE,
                    )
                    if k0 >= q0:
                        nc.vector.tensor_mul(
                            pt[:, lo : lo + 128],
                            pt[:, lo : lo + 128],
                            mask[:, :],
                        )
                    pts[(qh, j0)] = (pt, 0)
                else:
                    # packed pair: both blocks are diagonal-region blocks
                    # whose spans live entirely in the second 512-chunk
                    off = 0
                    shifts = []
                    for j in (j0, j0 + 1):
                        k0 = 128 * j
                        lo = k0 - q0  # >= 512 by construction
                        w = HALF - lo
                        nc.tensor.matmul(
                            sps[:, off : off + w],
                            kt[:, k0 : k0 + 128],
                            qt[:, q0 + lo : q0 + HALF],
                            start=True,
                            stop=True,
                        )
                        shifts.append(lo - off)
                        off += w
                    nc.scalar.activation(
                        pt[:, 0:off],
                        sps[:, 0:off],
                        mybir.ActivationFunctionType.Exp,
                        scale=SCALE,
                    )
                    off = 0
                    for idx, j in enumerate((j0, j0 + 1)):
                        nc.vector.tensor_mul(
                            pt[:, off : off + 128],
                            pt[:, off : off + 128],
                            mask[:, :],
                        )
                        pts[(qh, j)] = (pt, shifts[idx])
                        off += HALF - (128 * j - q0)

            # per-pass score-group lists: singles then two packed pairs
            def make_groups(qh):
                njb = (qh * HALF + HALF) // 128
                return ([(qh, j, 1) for j in range(njb - 4)]
                        + [(qh, njb - 4, 2), (qh, njb - 2, 2)])

            groups_all = make_groups(0) + make_groups(1)
            gcur = 0  # next group to emit
            blocks_emitted = 0

            def emit_through(nblocks):
                """Emit score groups until >= nblocks blocks are out."""
                nonlocal gcur, blocks_emitted
                while blocks_emitted < nblocks and gcur < len(groups_all):
                    g = groups_all[gcur]
                    emit_group(g[0], g[1], g[2])
                    blocks_emitted += g[2]
                    gcur += 1

            emit_through(2)
            # two fill-bubble warm matmuls: the PE would otherwise idle
            # ~1us here (scores of block 2 wait for exp(0) to free its
            # PSUM buffer) — a >1us gap restarts the HAM clock ramp
            for _w in range(2):
                nc.tensor.matmul(
                    warm_rs[:, :],
                    pe_scr[:, 0:128],
                    pe_scr[:, 0:512],
                    start=True,
                    stop=True,
                )

            for qh in range(2):
                q0 = qh * HALF  # global q offset of this pass
                njb = (q0 + HALF) // 128  # key blocks this pass

                out_ps = opool.tile([D, HALF], F32, tag="outps",
                                    name=f"outps_{qh}")
                rs = [
                    rpool.tile([128, 512], F32, tag="rs", name=f"rs_{qh}_{h}")
                    for h in range(2)
                ]
                # last key block / pair that touches each 512-half
                j_last = [(q0 + 512 * (h + 1)) // 128 - 1 for h in range(2)]
                m_last = [(q0 + 512 * (h + 1)) // 256 - 1 for h in range(2)]

                def emit_pv(j, q0=q0, j_last=j_last, out_ps=out_ps):
                    """PV accumulation for key block j."""
                    k0 = 128 * j
                    pt, shift = pts[(qh, j)]
                    q_lo = max(q0, k0)
                    for h in range(2):
                        a = max(q_lo, q0 + 512 * h)
                        b = q0 + 512 * (h + 1)
                        if a >= b:
                            continue
                        nc.tensor.matmul(
                            out_ps[:, a - q0 : b - q0],
                            vs[:, k0 : k0 + 128],
                            pt[:, a - q0 - shift : b - q0 - shift],
                            start=(j == 0),
                            stop=(j == j_last[h]),
                        )

                def emit_pair_add(m, q0=q0):
                    """pt_even += pt_odd over the causal overlap (VectorE,
                    bf16).  Runs after PV consumed pt_even."""
                    pa, sa = pts[(qh, 2 * m)]
                    pb, sb = pts[(qh, 2 * m + 1)]
                    lo = max(q0, 128 * (2 * m + 1)) - q0
                    nc.vector.tensor_add(
                        pa[:, lo - sa : HALF - sa],
                        pa[:, lo - sa : HALF - sa],
                        pb[:, lo - sb : HALF - sb],
                    )

                def emit_rowsum(m, q0=q0, rs=rs, m_last=m_last):
                    """Stream the pair-sum through the ones stationary into
                    the PSUM rowsum accumulator (per 512-chunk)."""
                    pa, sa = pts[(qh, 2 * m)]
                    su = max(q0, 256 * m)
                    for h in range(2):
                        a = max(su, q0 + 512 * h)
                        b = q0 + 512 * (h + 1)
                        if a >= b:
                            continue
                        nc.tensor.matmul(
                            rs[h][:, a - (q0 + 512 * h) : b - (q0 + 512 * h)],
                            ones_b[:, :],
                            pa[:, a - q0 - sa : b - q0 - sa],
                            start=(m == 0),
                            stop=(m == m_last[h]),
                        )

                def emit_epi_half(h, qh=qh, q0=q0, rs=rs, out_ps=out_ps):
                    """Normalize + store q-chunk [q0+512h, q0+512h+512).
                    Multiplies straight out of PSUM — no evacuation copy."""
                    rb = epi.tile([128, 512], F32, tag="rb",
                                  name=f"rb_{qh}_{h}")
                    o_fin = epi.tile([D, 512], F32, tag="o_fin",
                                     name=f"ofin_{qh}_{h}")
                    if qh == 1 and h == 1:
                        # final tail: (384,128) split — the 128-wide second
                        # piece makes the last store issue as early as
                        # possible; the two issues ride different
                        # hardware-DGE queues (ScalarE is idle by now)
                        splits = [(0, 384, nc.scalar), (384, 512, nc.sync)]
                    else:
                        splits = [(0, 512, nc.sync)]
                    for (a, b, eng) in splits:
                        nc.vector.reciprocal_approx_fast(
                            out=rb[:, a:b], in_=rs[h][:, a:b]
                        )
                        nc.vector.tensor_mul(
                            o_fin[:, a:b],
                            out_ps[:, 512 * h + a : 512 * h + b],
                            rb[:, a:b],
                        )
                        if b - a > 256:
                            for c0 in range(a, b, 256):
                                c1 = min(c0 + 256, b)
                                eng.dma_start(
                                    out_d[:, q0 + 512 * h + c0 :
                                          q0 + 512 * h + c1],
                                    o_fin[:, c0:c1],
                                )
                        else:
                            eng.dma_start(
                                out_d[:, q0 + 512 * h + a : q0 + 512 * h + b],
                                o_fin[:, a:b],
                            )

                # software pipeline: keep the score stream two blocks ahead
                # of the PV/rowsum consumers, continued ACROSS the pass
                # boundary
                for j in range(njb):
                    emit_through(8 * qh + j + 3)
                    emit_pv(j)
                    if j % 2 == 1:
                        m = j // 2
                        emit_pair_add(m)
                        emit_rowsum(m)
                        pts.pop((qh, 2 * m))
                        pts.pop((qh, 2 * m + 1))
                    for h in range(2):
                        if j == j_last[h]:
                            emit_epi_half(h)

    nc.compile()
    return nc


def _get_nc():
    global _NC_CACHE
    if _NC_CACHE is None:
        _NC_CACHE = _build_nc()
    return _NC_CACHE


def _in_maps(Q, K, V):
    maps = []
    for b in range(B):
        vsb = np.ascontiguousarray(
            V[b].reshape(NBLK, 128, D).transpose(1, 0, 2).reshape(128, S)
        ).astype(BF16_NP)
        maps.append(
            {
                "QT": np.ascontiguousarray(Q[b].T).astype(BF16_NP),
                "KT": np.ascontiguousarray(K[b].T).astype(BF16_NP),
                "VS": vsb,
            }
        )
    return maps


def kernel(Q, K, V):
    Q = np.asarray(Q, dtype=np.float32)
    K = np.asarray(K, dtype=np.float32)
    V = np.asarray(V, dtype=np.float32)
    assert Q.shape == (B, S, D), Q.shape

    nc = _get_nc()
    res = run_bass_kernel_spmd(nc, _in_maps(Q, K, V), core_ids=list(range(B)))
    return np.stack(
        [np.ascontiguousarray(res.results[b]["out"].T) for b in range(B)], axis=0
    )
